# revision 11
# baseline (speedup 1.0000x reference)
"""Trainium2 Bass kernel for nn_LossStyle (VGG nn-style loss), 8-core SPMD.

Each core computes an eighth (12-row strip of the 96x96 grid, halo recompute)
of VGG features for BOTH images (bf16 activations, fp32 conv3 outputs);
AllGathers: style channel sums; centered+normalized style features (bf16);
per-style-column stats (fp32); output channel sums. Each core then computes
its [1152, 9216] similarity slab on TensorE (pre-normalized operands so PSUM
holds the argmax objective), argmax via max/max_index, indirect-DMA stat
gather, and reconstructs the cosine loss from per-column scalars.
"""
import numpy as np
import ml_dtypes

N = 8  # cores
BF16 = ml_dtypes.bfloat16

MEAN = np.array([0.485, 0.456, 0.406], dtype=np.float32)
STD = np.array([0.229, 0.224, 0.225], dtype=np.float32)

_CACHE = {}


def _host_prep(inputs):
    f32 = np.float32
    w0n = np.asarray(inputs['w0'], f32)
    b0n = np.asarray(inputs['b0'], f32)

    def t9(w):  # [O, C, 3, 3] -> [9, C, O]
        return np.ascontiguousarray(w.transpose(2, 3, 1, 0).reshape(9, w.shape[1], w.shape[0]))

    w1bd = np.zeros((54, 128), f32)
    for t in range(9):
        kh, kw = divmod(t, 3)
        for c in range(3):
            w1bd[t * 3 + c, 0:64] = w0n[:, c, kh, kw]
            w1bd[27 + t * 3 + c, 64:128] = w0n[:, c, kh, kw]

    w1r = t9(np.asarray(inputs['w1'], f32))
    w2bd = np.zeros((128, 9, 128), f32)
    w2bd[0:64, :, 0:64] = w1r.transpose(1, 0, 2)
    w2bd[64:128, :, 64:128] = w1r.transpose(1, 0, 2)

    w2r = t9(np.asarray(inputs['w2'], f32))
    w3bd = np.zeros((128, 9, 256), f32)
    w3bd[0:64, :, 0:128] = w2r.transpose(1, 0, 2)
    w3bd[64:128, :, 128:256] = w2r.transpose(1, 0, 2)

    w4bd = np.ascontiguousarray(t9(np.asarray(inputs['w3'], f32)).transpose(1, 0, 2))
    w5bd = np.ascontiguousarray(t9(np.asarray(inputs['w4'], f32)).transpose(1, 0, 2))

    def big(wref):  # [256, 256, 3, 3] -> [128, 9, 2, 256]
        w = t9(np.asarray(wref, f32))
        out = np.zeros((128, 9, 2, 256), f32)
        for kb in range(2):
            out[:, :, kb, :] = w[:, kb * 128:(kb + 1) * 128, :].transpose(1, 0, 2)
        return out

    w6bd = big(inputs['w5'])
    w7bd = big(inputs['w6'])

    b1 = np.concatenate([b0n, b0n]).astype(f32)
    b1_ = np.asarray(inputs['b1'], f32)
    b2 = np.concatenate([b1_, b1_]).astype(f32)

    weights = dict(
        w1bd=w1bd.astype(BF16), w2bd=w2bd.astype(BF16), w3bd=w3bd.astype(BF16),
        w4bd=w4bd.astype(BF16), w5bd=w5bd.astype(BF16), w6bd=w6bd, w7bd=w7bd,
        bias1=b1, bias2=b2,
        bias3=np.asarray(inputs['b2'], f32), bias4=np.asarray(inputs['b3'], f32),
        bias5=np.asarray(inputs['b4'], f32).reshape(2, 128).T.copy(),
        bias6=np.asarray(inputs['b5'], f32).reshape(2, 128).T.copy(),
        bias7=np.asarray(inputs['b6'], f32).reshape(2, 128).T.copy())

    def slabs(img):
        imgn = (img - MEAN[:, None, None]) / STD[:, None, None]
        out = []
        for k in range(N):
            s = np.zeros((3, 86, 386), f32)
            g0 = 48 * k - 19
            lo, hi = max(0, g0), min(384, g0 + 86)
            s[:, lo - g0:hi - g0, 1:385] = imgn[:, lo:hi, :]
            out.append(s.astype(BF16))
        return out

    xs = slabs(np.asarray(inputs['outputs'], f32)[0])
    ss = slabs(np.asarray(inputs['styles'], f32)[0])
    in_maps = []
    for k in range(N):
        m = dict(weights)
        m['xslab'] = xs[k]
        m['sslab'] = ss[k]
        in_maps.append(m)
    return in_maps


def build_kernel(debug=False):
    import concourse.bass as bass
    import concourse.bacc as bacc
    import concourse.mybir as mybir
    import concourse.tile as tile

    dt = mybir.dt
    ALU = mybir.AluOpType
    ACT = mybir.ActivationFunctionType
    AX = mybir.AxisListType

    nc = bacc.Bacc("TRN2", target_bir_lowering=False, debug=False, num_devices=N)

    xslab = nc.dram_tensor("xslab", [3, 86, 386], dt.bfloat16, kind="ExternalInput")
    sslab = nc.dram_tensor("sslab", [3, 86, 386], dt.bfloat16, kind="ExternalInput")
    wd = {}
    wd[1] = nc.dram_tensor("w1bd", [54, 128], dt.bfloat16, kind="ExternalInput")
    wd[2] = nc.dram_tensor("w2bd", [128, 9, 128], dt.bfloat16, kind="ExternalInput")
    wd[3] = nc.dram_tensor("w3bd", [128, 9, 256], dt.bfloat16, kind="ExternalInput")
    wd[4] = nc.dram_tensor("w4bd", [128, 9, 128], dt.bfloat16, kind="ExternalInput")
    wd[5] = nc.dram_tensor("w5bd", [128, 9, 256], dt.bfloat16, kind="ExternalInput")
    wd[6] = nc.dram_tensor("w6bd", [128, 9, 2, 256], dt.float32, kind="ExternalInput")
    wd[7] = nc.dram_tensor("w7bd", [128, 9, 2, 256], dt.float32, kind="ExternalInput")
    bias_d = {}
    for i, shp in ((1, [128]), (2, [128]), (3, [128]), (4, [128]),
                   (5, [128, 2]), (6, [128, 2]), (7, [128, 2])):
        bias_d[i] = nc.dram_tensor(f"bias{i}", shp, dt.float32, kind="ExternalInput")

    loss_parts = nc.dram_tensor("loss_parts", [128, 9], dt.float32, kind="ExternalOutput")
    idx_out = nc.dram_tensor("idx_out", [128, 9], dt.uint32, kind="ExternalOutput")
    if debug:
        dbg_feats = nc.dram_tensor("dbg_feats", [128, 6912], dt.float32, kind="ExternalOutput")
        dbg_maxv = nc.dram_tensor("dbg_maxv", [128, 9], dt.float32, kind="ExternalOutput")
        dbg_stat = nc.dram_tensor("dbg_stat", [128, 27], dt.float32, kind="ExternalOutput")

    RG = [list(range(N))]
    BD = dt.bfloat16

    with tile.TileContext(nc) as tc:
        with tc.tile_pool(name="sb", bufs=1) as sb, \
             tc.tile_pool(name="wsl", bufs=2) as wsl, \
             tc.tile_pool(name="cv", bufs=1) as cv, \
             tc.tile_pool(name="sh", bufs=2) as sh, \
             tc.tile_pool(name="pp", bufs=2) as pp, \
             tc.tile_pool(name="ps", bufs=4, space="PSUM") as ps, \
             tc.tile_pool(name="ps2", bufs=1, space="PSUM") as ps2, \
             tc.tile_pool(name="dram", bufs=1, space="DRAM") as dram:

            ones = sb.tile([128, 128], dt.float32, tag="ones")
            nc.vector.memset(ones[:], 1.0)
            zero1 = sb.tile([128, 1], dt.float32, tag="zero1")
            nc.vector.memset(zero1[:], 0.0)
            eps1 = sb.tile([128, 1], dt.float32, tag="eps1")
            nc.vector.memset(eps1[:], 1e-8)
            partS = sb.tile([128, 6], dt.float32, tag="partS")
            partX = sb.tile([128, 6], dt.float32, tag="partX")

            biases = {}
            for i in range(1, 8):
                shp = [128, 2] if i >= 5 else [128, 1]
                t = sb.tile(shp, dt.float32, tag=f"b{i}", name=f"b{i}")
                nc.sync.dma_start(t[:], bias_d[i][:])
                biases[i] = t

            state = {}

            def conv_image(slab_dram, im, part_tile):
                i2c = cv.tile([54, 42, 384], BD, tag="A", name="i2c")
                for t in range(9):
                    kh, kw = divmod(t, 3)
                    for half in range(2):
                        src = slab_dram[:, kh + 42 * half: kh + 42 * half + 42, kw:kw + 384]
                        nc.sync.dma_start(i2c[27 * half + 3 * t: 27 * half + 3 * t + 3, :, :], src)
                w1 = wsl.tile([54, 128], BD, tag="w", name="w1")
                nc.sync.dma_start(w1[:], wd[1][:])

                c11 = cv.tile([128, 44, 386], BD, tag="B", name="c11")
                nc.vector.memset(c11[0:64, 0:1, :], 0.0)
                nc.vector.memset(c11[64:128, 43:44, :], 0.0)
                nc.vector.memset(c11[:, :, 0:1], 0.0)
                nc.vector.memset(c11[:, :, 385:386], 0.0)
                for r in range(42):
                    p = ps.tile([128, 384], dt.float32, tag="mm", name="p11")
                    nc.tensor.matmul(p[:], w1[:], i2c[:, r, :], start=True, stop=True)
                    nc.scalar.activation(c11[:, r + 1, 1:385], p[:], ACT.Relu,
                                         bias=biases[1][:, 0:1])
                nc.sync.dma_start(c11[0:64, 43:44, :], c11[64:128, 1:2, :])
                nc.sync.dma_start(c11[64:128, 0:1, :], c11[0:64, 42:43, :])

                w2 = wsl.tile([128, 9, 128], BD, tag="w", name="w2")
                nc.sync.dma_start(w2[:], wd[2][:])
                c12 = cv.tile([128, 42, 386], BD, tag="A", name="c12")
                nc.vector.memset(c12[:, :, 0:1], 0.0)
                nc.vector.memset(c12[:, :, 385:386], 0.0)
                for v in range(42):
                    p = ps.tile([128, 384], dt.float32, tag="mm", name="p12")
                    for t in range(9):
                        kh, kw = divmod(t, 3)
                        nc.tensor.matmul(p[:], w2[:, t, :], c11[:, v + kh, kw:kw + 384],
                                         start=(t == 0), stop=(t == 8))
                    nc.scalar.activation(c12[:, v, 1:385], p[:], ACT.Relu,
                                         bias=biases[2][:, 0:1])

                p1t = cv.tile([128, 42, 192], BD, tag="B", name="p1t")
                nc.vector.tensor_tensor(p1t[:], c12[:, :, 1:385:2], c12[:, :, 2:386:2], op=ALU.max)
                p1o = cv.tile([128, 21, 192], BD, tag="A", name="p1o")
                nc.vector.tensor_tensor(p1o[:], p1t[:, 0:42:2, :], p1t[:, 1:42:2, :], op=ALU.max)

                c21 = cv.tile([128, 22, 194], BD, tag="B", name="c21")
                nc.vector.memset(c21[:, :, 0:1], 0.0)
                nc.vector.memset(c21[:, :, 193:194], 0.0)
                nc.vector.tensor_copy(c21[0:64, 0:21, 1:193], p1o[0:64, :, :])
                nc.vector.tensor_copy(c21[64:128, 1:22, 1:193], p1o[64:128, :, :])
                nc.sync.dma_start(c21[0:64, 21:22, 1:193], p1o[64:128, 0:1, :])
                nc.sync.dma_start(c21[64:128, 0:1, 1:193], p1o[0:64, 20:21, :])

                w3 = wsl.tile([128, 9, 256], BD, tag="w", name="w3")
                nc.sync.dma_start(w3[:], wd[3][:])
                c22i = cv.tile([128, 42, 194], BD, tag="A", name="c22i")
                nc.vector.memset(c22i[:, 0:1, :], 0.0)
                nc.vector.memset(c22i[:, 41:42, :], 0.0)
                nc.vector.memset(c22i[:, :, 0:1], 0.0)
                nc.vector.memset(c22i[:, :, 193:194], 0.0)
                for mb in range(2):
                    for q0 in range(0, 20, 2):
                        p = ps.tile([128, 2, 192], dt.float32, tag="mm", name="p21")
                        for t in range(9):
                            kh, kw = divmod(t, 3)
                            nc.tensor.matmul(p[:], w3[:, t, 128 * mb:128 * mb + 128],
                                             c21[:, q0 + kh:q0 + kh + 2, kw:kw + 192],
                                             start=(t == 0), stop=(t == 8))
                        w_ = 20 * mb + q0 + 1
                        nc.scalar.activation(c22i[:, w_:w_ + 2, 1:193], p[:], ACT.Relu,
                                             bias=biases[3][:, 0:1])

                w4 = wsl.tile([128, 9, 128], BD, tag="w", name="w4")
                nc.sync.dma_start(w4[:], wd[4][:])
                c22o = cv.tile([128, 40, 194], BD, tag="B", name="c22o")
                nc.vector.memset(c22o[:, :, 0:1], 0.0)
                nc.vector.memset(c22o[:, :, 193:194], 0.0)
                for q0 in range(0, 40, 2):
                    p = ps.tile([128, 2, 192], dt.float32, tag="mm", name="p22")
                    for t in range(9):
                        kh, kw = divmod(t, 3)
                        nc.tensor.matmul(p[:], w4[:, t, :],
                                         c22i[:, q0 + kh:q0 + kh + 2, kw:kw + 192],
                                         start=(t == 0), stop=(t == 8))
                    nc.scalar.activation(c22o[:, q0:q0 + 2, 1:193], p[:], ACT.Relu,
                                         bias=biases[4][:, 0:1])

                p2t = cv.tile([128, 40, 96], BD, tag="A", name="p2t")
                nc.vector.tensor_tensor(p2t[:], c22o[:, :, 1:193:2], c22o[:, :, 2:194:2], op=ALU.max)
                c31 = cv.tile([128, 22, 98], BD, tag="B", name="c31")
                nc.vector.memset(c31[:, 0:1, :], 0.0)
                nc.vector.memset(c31[:, 21:22, :], 0.0)
                nc.vector.memset(c31[:, :, 0:1], 0.0)
                nc.vector.memset(c31[:, :, 97:98], 0.0)
                nc.vector.tensor_tensor(c31[:, 1:21, 1:97], p2t[:, 0:40:2, :],
                                        p2t[:, 1:40:2, :], op=ALU.max)

                w5 = wsl.tile([128, 9, 256], BD, tag="w", name="w5")
                nc.sync.dma_start(w5[:], wd[5][:])
                f11 = cv.tile([128, 2, 22, 98], dt.float32, tag="C", name="f11")
                for kb in range(2):
                    nc.vector.memset(f11[:, kb, 0:1, :], 0.0)
                    nc.vector.memset(f11[:, kb, 21:22, :], 0.0)
                    nc.vector.memset(f11[:, kb, :, 0:1], 0.0)
                    nc.vector.memset(f11[:, kb, :, 97:98], 0.0)
                for mb in range(2):
                    for h0 in range(0, 20, 5):
                        p = ps.tile([128, 5, 96], dt.float32, tag="mm", name="p31")
                        for t in range(9):
                            kh, kw = divmod(t, 3)
                            nc.tensor.matmul(p[:], w5[:, t, 128 * mb:128 * mb + 128],
                                             c31[:, h0 + kh:h0 + kh + 5, kw:kw + 96],
                                             start=(t == 0), stop=(t == 8))
                        nc.scalar.activation(f11[:, mb, h0 + 1:h0 + 6, 1:97], p[:], ACT.Relu,
                                             bias=biases[5][:, mb:mb + 1])

                w6 = wsl.tile([128, 9, 2, 256], dt.float32, tag="w", name="w6")
                nc.sync.dma_start(w6[:], wd[6][:])
                f13 = cv.tile([128, 2, 22, 98], dt.float32, tag="A", name="f13")
                for kb in range(2):
                    nc.vector.memset(f13[:, kb, 0:1, :], 0.0)
                    nc.vector.memset(f13[:, kb, 21:22, :], 0.0)
                    nc.vector.memset(f13[:, kb, :, 0:1], 0.0)
                    nc.vector.memset(f13[:, kb, :, 97:98], 0.0)
                for mb in range(2):
                    for h0 in range(0, 20, 5):
                        p = ps.tile([128, 5, 96], dt.float32, tag="mm", name="p32")
                        first = True
                        for kb in range(2):
                            for t in range(9):
                                kh, kw = divmod(t, 3)
                                nc.tensor.matmul(p[:], w6[:, t, kb, 128 * mb:128 * mb + 128],
                                                 f11[:, kb, h0 + kh:h0 + kh + 5, kw:kw + 96],
                                                 start=first, stop=(kb == 1 and t == 8))
                                first = False
                        nc.scalar.activation(f13[:, mb, h0 + 1:h0 + 6, 1:97], p[:], ACT.Relu,
                                             bias=biases[6][:, mb:mb + 1])

                w7 = wsl.tile([128, 9, 2, 256], dt.float32, tag="w", name="w7")
                nc.sync.dma_start(w7[:], wd[7][:])
                f15 = cv.tile([128, 2, 12, 96], dt.float32, tag="B", name="f15")
                for mb in range(2):
                    for h0, nr in ((4, 5), (9, 5), (14, 2)):
                        p = ps.tile([128, 5, 96], dt.float32, tag="mm", name="p33")
                        first = True
                        for kb in range(2):
                            for t in range(9):
                                kh, kw = divmod(t, 3)
                                nc.tensor.matmul(p[:, 0:nr, :],
                                                 w7[:, t, kb, 128 * mb:128 * mb + 128],
                                                 f13[:, kb, h0 + kh:h0 + kh + nr, kw:kw + 96],
                                                 start=first, stop=(kb == 1 and t == 8))
                                first = False
                        nc.scalar.activation(f15[:, mb, h0 - 4:h0 - 4 + nr, :], p[:, 0:nr, :],
                                             ACT.Relu, bias=biases[7][:, mb:mb + 1])

                wins = {0: f11[:, 0, 5:17, 1:97], 1: f11[:, 1, 5:17, 1:97],
                        2: f13[:, 0, 5:17, 1:97], 3: f13[:, 1, 5:17, 1:97],
                        4: f15[:, 0, :, :], 5: f15[:, 1, :, :]}
                if im == 0:
                    feat0 = sh.tile([128, 6912], dt.float32, tag="shA", name="feat0")
                    state['feat0'] = feat0
                    for b in range(6):
                        nc.vector.tensor_copy(
                            feat0[:, b * 1152:(b + 1) * 1152].rearrange("p (r x) -> p r x", r=12),
                            wins[b])
                        nc.vector.tensor_reduce(part_tile[:, b:b + 1],
                                                feat0[:, b * 1152:(b + 1) * 1152],
                                                axis=AX.X, op=ALU.add)
                else:
                    state['xwin'] = wins
                    for b in range(6):
                        nc.vector.tensor_reduce(part_tile[:, b:b + 1], wins[b],
                                                axis=AX.XY, op=ALU.add)

            # ======== styles conv, style AGs, then x conv (overlaps AG2) ========
            conv_image(sslab, 0, partS)

            cc1s_in = dram.tile([128, 6], dt.float32)
            cc1s_out = dram.tile([128 * N, 6], dt.float32, addr_space="Shared")
            nc.sync.dma_start(cc1s_in[:], partS[:])
            nc.gpsimd.collective_compute("AllGather", ALU.bypass, replica_groups=RG,
                                         ins=[cc1s_in.opt()], outs=[cc1s_out.opt()])
            sumS = sb.tile([128, 48], dt.float32, tag="sums", name="sumS")
            nc.sync.dma_start(sumS[:].rearrange("p (r c) -> p r c", r=8),
                              cc1s_out[:].rearrange("(r p) c -> p r c", p=128))
            bmean = sb.tile([128, 6], dt.float32, tag="bmean")
            for col in range(6):
                nc.vector.tensor_reduce(bmean[:, col:col + 1], sumS[:, col:48:6],
                                        axis=AX.X, op=ALU.add)
            nc.scalar.mul(bmean[:], bmean[:], 1.0 / 9216.0)

            Fs = state['feat0']
            for b in range(6):
                nc.vector.tensor_scalar(Fs[:, b * 1152:(b + 1) * 1152],
                                        Fs[:, b * 1152:(b + 1) * 1152],
                                        bmean[:, b:b + 1], None, op0=ALU.subtract)
            nbc_t = sb.tile([128, 1152], dt.float32, tag="nbc_u", name="nbc_t")
            inv_t = sb.tile([128, 1152], dt.float32, tag="inv_na", name="inv_t")
            cc3_in = dram.tile([1152, 3], dt.float32)
            cc3_out = dram.tile([1152 * N, 3], dt.float32, addr_space="Shared")
            for ch in range(3):
                c0 = ch * 384
                p_s2 = ps2.tile([128, 384], dt.float32, tag="p_s2", name="ps2s")
                p_pm = ps2.tile([128, 384], dt.float32, tag="p_pm", name="ppms")
                for b in range(6):
                    sq = pp.tile([128, 384], dt.float32, tag="sq", bufs=1, name="sqs")
                    nc.scalar.activation(sq[:], Fs[:, b * 1152 + c0:b * 1152 + c0 + 384],
                                         ACT.Square, bias=zero1[:])
                    rep = pp.tile([128, 128], dt.float32, tag="rep", name="reps")
                    nc.vector.tensor_scalar(rep[:], ones[:], bmean[:, b:b + 1], None, op0=ALU.mult)
                    nc.tensor.matmul(p_s2[:], ones[:], sq[:], start=(b == 0), stop=(b == 5))
                    nc.tensor.matmul(p_pm[:], rep[:], Fs[:, b * 1152 + c0:b * 1152 + c0 + 384],
                                     start=(b == 0), stop=(b == 5))
                nc.scalar.activation(nbc_t[:, c0:c0 + 384], p_s2[:], ACT.Sqrt, bias=eps1[:])
                nc.vector.tensor_scalar(nbc_t[:, c0:c0 + 384], nbc_t[:, c0:c0 + 384],
                                        1e-8, None, op0=ALU.add)
                nc.vector.reciprocal(inv_t[:, c0:c0 + 384], nbc_t[:, c0:c0 + 384])
                pmcp = pp.tile([128, 384], dt.float32, tag="sq", bufs=1, name="pmcp")
                nc.scalar.copy(pmcp[:], p_pm[:])
                nc.sync.dma_start(cc3_in[c0:c0 + 384, 1:2], pmcp[0:1, :])
            nc.sync.dma_start(cc3_in[:, 0:1], nbc_t[0:1, :])  # sbuf row -> strided dram

            bhat = sb.tile([128, 6912], dt.bfloat16, tag="cast16", name="bhat")
            for b in range(6):
                nc.vector.tensor_tensor(bhat[:, b * 1152:(b + 1) * 1152],
                                        Fs[:, b * 1152:(b + 1) * 1152], inv_t[:], op=ALU.mult)
            cc2_in = dram.tile([768, 1152], dt.bfloat16)
            cc2_out = dram.tile([768 * N, 1152], dt.bfloat16, addr_space="Shared")
            nc.sync.dma_start(cc2_in[:].rearrange("(b p) x -> p b x", p=128),
                              bhat[:].rearrange("p (b x) -> p b x", b=6))
            nc.gpsimd.collective_compute("AllGather", ALU.bypass, replica_groups=RG,
                                         ins=[cc2_in.opt()], outs=[cc2_out.opt()])

            # ======== x conv (overlaps AG2) ========
            conv_image(xslab, 1, partX)

            cc1x_in = dram.tile([128, 6], dt.float32)
            cc1x_out = dram.tile([128 * N, 6], dt.float32, addr_space="Shared")
            nc.sync.dma_start(cc1x_in[:], partX[:])
            nc.gpsimd.collective_compute("AllGather", ALU.bypass, replica_groups=RG,
                                         ins=[cc1x_in.opt()], outs=[cc1x_out.opt()])
            sumX = sb.tile([128, 48], dt.float32, tag="sums", name="sumX")
            nc.sync.dma_start(sumX[:].rearrange("p (r c) -> p r c", r=8),
                              cc1x_out[:].rearrange("(r p) c -> p r c", p=128))
            amean = sb.tile([128, 6], dt.float32, tag="amean")
            for col in range(6):
                nc.vector.tensor_reduce(amean[:, col:col + 1], sumX[:, col:48:6],
                                        axis=AX.X, op=ALU.add)
            nc.scalar.mul(amean[:], amean[:], 1.0 / 9216.0)

            psc_a = ps2.tile([1, 1], dt.float32, tag="p_s2", name="psc_a")
            psc_b = ps2.tile([1, 1], dt.float32, tag="p_pm", name="psc_b")
            psc_w = ps2.tile([1, 1], dt.float32, tag="p_v", name="psc_w")
            for b in range(6):
                st, sp = (b == 0), (b == 5)
                nc.tensor.matmul(psc_a[:], amean[:, b:b + 1], amean[:, b:b + 1], start=st, stop=sp)
                nc.tensor.matmul(psc_b[:], bmean[:, b:b + 1], bmean[:, b:b + 1], start=st, stop=sp)
                nc.tensor.matmul(psc_w[:], amean[:, b:b + 1], bmean[:, b:b + 1], start=st, stop=sp)
            scal = sb.tile([1, 3], dt.float32, tag="scal")
            nc.scalar.copy(scal[:, 0:1], psc_a[:])
            nc.scalar.copy(scal[:, 1:2], psc_b[:])
            nc.scalar.copy(scal[:, 2:3], psc_w[:])
            pscr = ps2.tile([128, 3], dt.float32, tag="p_s2", name="pscr")
            nc.tensor.matmul(pscr[:], ones[0:1, :], scal[:], start=True, stop=True)
            screp = sb.tile([128, 3], dt.float32, tag="screp")
            nc.scalar.copy(screp[:], pscr[:])  # A2, B2, W

            for ch in range(3):
                c0 = ch * 384
                p_v = ps2.tile([128, 384], dt.float32, tag="p_v", name="pvs")
                for b in range(6):
                    rep = pp.tile([128, 128], dt.float32, tag="rep", name="repv")
                    nc.vector.tensor_scalar(rep[:], ones[:], amean[:, b:b + 1], None, op0=ALU.mult)
                    nc.tensor.matmul(p_v[:], rep[:], Fs[:, b * 1152 + c0:b * 1152 + c0 + 384],
                                     start=(b == 0), stop=(b == 5))
                pvcp = pp.tile([128, 384], dt.float32, tag="sq", bufs=1, name="pvcp")
                nc.scalar.copy(pvcp[:], p_v[:])
                nc.sync.dma_start(cc3_in[c0:c0 + 384, 2:3], pvcp[0:1, :])
            nc.gpsimd.collective_compute("AllGather", ALU.bypass, replica_groups=RG,
                                         ins=[cc3_in.opt()], outs=[cc3_out.opt()])

            # ======== x side: center + stats (windowed) + bf16 cast ========
            xwin = state['xwin']
            for b in range(6):
                nc.vector.tensor_scalar(xwin[b], xwin[b], amean[:, b:b + 1], None,
                                        op0=ALU.subtract)
            acb = sb.tile([128, 6912], dt.bfloat16, tag="cast16", name="acb")
            for b in range(6):
                nc.vector.tensor_copy(
                    acb[:, b * 1152:(b + 1) * 1152].rearrange("p (r x) -> p r x", r=12),
                    xwin[b])
            if debug:
                nc.sync.dma_start(dbg_feats[:], Fs[:])

            na_t = sb.tile([128, 1152], dt.float32, tag="inv_na", name="na_t")
            u_t = sb.tile([128, 1152], dt.float32, tag="nbc_u", name="u_t")
            for ch in range(3):
                c0 = ch * 384
                p_s2 = ps2.tile([128, 384], dt.float32, tag="p_s2", name="ps2x")
                p_pm = ps2.tile([128, 384], dt.float32, tag="p_pm", name="ppmx")
                p_v = ps2.tile([128, 384], dt.float32, tag="p_v", name="pvx")
                for b in range(6):
                    win = xwin[b][:, 4 * ch:4 * ch + 4, :]
                    sq = pp.tile([128, 384], dt.float32, tag="sq", bufs=1, name="sqx")
                    nc.scalar.activation(sq[:], win, ACT.Square, bias=zero1[:])
                    repa = pp.tile([128, 128], dt.float32, tag="rep", name="repa")
                    nc.vector.tensor_scalar(repa[:], ones[:], amean[:, b:b + 1], None, op0=ALU.mult)
                    repb = pp.tile([128, 128], dt.float32, tag="rep", name="repb")
                    nc.vector.tensor_scalar(repb[:], ones[:], bmean[:, b:b + 1], None, op0=ALU.mult)
                    nc.tensor.matmul(p_s2[:], ones[:], sq[:], start=(b == 0), stop=(b == 5))
                    nc.tensor.matmul(p_pm[:], repa[:], win, start=(b == 0), stop=(b == 5))
                    nc.tensor.matmul(p_v[:], repb[:], win, start=(b == 0), stop=(b == 5))
                nc.vector.tensor_scalar(na_t[:, c0:c0 + 384], p_pm[:], 2.0, None, op0=ALU.mult)
                nc.vector.tensor_tensor(na_t[:, c0:c0 + 384], na_t[:, c0:c0 + 384], p_s2[:],
                                        op=ALU.add)
                nc.vector.tensor_scalar(na_t[:, c0:c0 + 384], na_t[:, c0:c0 + 384],
                                        screp[:, 0:1], None, op0=ALU.add)
                nc.scalar.activation(na_t[:, c0:c0 + 384], na_t[:, c0:c0 + 384], ACT.Sqrt,
                                     bias=zero1[:])
                nc.scalar.copy(u_t[:, c0:c0 + 384], p_v[:])

            nat = sb.tile([128, 9], dt.float32, tag="nat")
            ut = sb.tile([128, 9], dt.float32, tag="ut")
            for m in range(9):
                nc.sync.dma_start(nat[:, m:m + 1], na_t[0:1, m * 128:(m + 1) * 128])
                nc.sync.dma_start(ut[:, m:m + 1], u_t[0:1, m * 128:(m + 1) * 128])

            # ======== cdist + argmax (stream bhat by column quarters) ========
            vmall = sb.tile([128, 36], dt.float32, tag="vmall")
            viall = sb.tile([128, 36], dt.float32, tag="viall")
            CH = (0, 512, 1024, 1536, 2048, 2304)
            for q in range(4):
                ballq = sh.tile([128, 6, 2304], dt.bfloat16, tag="shA", name="ballq")
                for r in range(2):
                    rows = (2 * q + r) * 768
                    nc.sync.dma_start(
                        ballq[:, :, r * 1152:(r + 1) * 1152],
                        cc2_out[rows:rows + 768, :].rearrange("(b p) x -> p b x", p=128))
                for m in range(9):
                    simq = wsl.tile([128, 2304], dt.float32, tag="w", name="simq")
                    for ci in range(5):
                        n0, n1 = CH[ci], CH[ci + 1]
                        p = ps.tile([128, 512], dt.float32, tag="mm", name="pcd")
                        for b in range(6):
                            nc.tensor.matmul(
                                p[:, 0:n1 - n0],
                                acb[:, b * 1152 + m * 128:b * 1152 + (m + 1) * 128],
                                ballq[:, b, n0:n1],
                                start=(b == 0), stop=(b == 5))
                        nc.scalar.copy(simq[:, n0:n1], p[:, 0:n1 - n0])
                    vm8 = pp.tile([128, 8], dt.float32, tag="vm8", name="vm8")
                    vi8 = pp.tile([128, 8], dt.uint32, tag="vi8", name="vi8")
                    nc.vector.max(vm8[:], simq[:])
                    nc.vector.max_index(vi8[:], vm8[:], simq[:])
                    nc.vector.tensor_copy(vmall[:, q * 9 + m:q * 9 + m + 1], vm8[:, 0:1])
                    nc.vector.tensor_copy(viall[:, q * 9 + m:q * 9 + m + 1], vi8[:, 0:1])

            best = sb.tile([128, 9], dt.float32, tag="best")
            bidx = sb.tile([128, 9], dt.float32, tag="bidx")
            nc.vector.tensor_copy(best[:], vmall[:, 0:9])
            nc.vector.tensor_copy(bidx[:], viall[:, 0:9])
            mq = sb.tile([128, 9], dt.uint8, tag="mq")
            iq = sb.tile([128, 9], dt.float32, tag="iq")
            for q in range(1, 4):
                nc.vector.tensor_scalar(iq[:], viall[:, q * 9:(q + 1) * 9],
                                        float(q * 2304), None, op0=ALU.add)
                nc.vector.tensor_tensor(mq[:], best[:], vmall[:, q * 9:(q + 1) * 9], op=ALU.is_ge)
                nc.vector.select(bidx[:], mq[:], bidx[:], iq[:])
                nc.vector.tensor_tensor(best[:], best[:], vmall[:, q * 9:(q + 1) * 9], op=ALU.max)
            idxs = sb.tile([128, 9], dt.uint32, tag="idxs")
            nc.vector.tensor_copy(idxs[:], bidx[:])
            nc.sync.dma_start(idx_out[:], idxs[:])

            statall = sb.tile([128, 27], dt.float32, tag="statall")
            for m in range(9):
                nc.gpsimd.indirect_dma_start(
                    out=statall[:, 3 * m:3 * m + 3], out_offset=None,
                    in_=cc3_out[:],
                    in_offset=bass.IndirectOffsetOnAxis(ap=idxs[:, m:m + 1], axis=0))
            if debug:
                nc.sync.dma_start(dbg_maxv[:], best[:])
                nc.sync.dma_start(dbg_stat[:], statall[:])

            # ======== loss assembly ========
            nbc_g = statall[:, 0:27:3]
            pmb_g = statall[:, 1:27:3]
            v_g = statall[:, 2:27:3]
            dotab = sb.tile([128, 9], dt.float32, tag="dotab")
            nc.vector.tensor_tensor(dotab[:], best[:], nbc_g, op=ALU.mult)
            nc.vector.tensor_tensor(dotab[:], dotab[:], ut[:], op=ALU.add)
            nc.vector.tensor_tensor(dotab[:], dotab[:], v_g, op=ALU.add)
            nc.vector.tensor_scalar(dotab[:], dotab[:], screp[:, 2:3], None, op0=ALU.add)
            nb = sb.tile([128, 9], dt.float32, tag="nb")
            nc.vector.tensor_scalar(nb[:], nbc_g, -1e-8, None, op0=ALU.add)
            nc.vector.tensor_tensor(nb[:], nb[:], nb[:], op=ALU.mult)
            nc.vector.tensor_scalar(nb[:], nb[:], -1e-8, None, op0=ALU.add)
            tmp9 = sb.tile([128, 9], dt.float32, tag="tmp9")
            nc.vector.tensor_scalar(tmp9[:], pmb_g, 2.0, None, op0=ALU.mult)
            nc.vector.tensor_tensor(nb[:], nb[:], tmp9[:], op=ALU.add)
            nc.vector.tensor_scalar(nb[:], nb[:], screp[:, 1:2], None, op0=ALU.add)
            nc.scalar.activation(nb[:], nb[:], ACT.Sqrt, bias=zero1[:])
            den = sb.tile([128, 9], dt.float32, tag="den")
            nc.vector.tensor_scalar(den[:], nat[:], 1e-8, None, op0=ALU.add)
            nc.vector.tensor_scalar(nb[:], nb[:], 1e-8, None, op0=ALU.add)
            nc.vector.tensor_tensor(den[:], den[:], nb[:], op=ALU.mult)
            nc.vector.reciprocal(den[:], den[:])
            loss9 = sb.tile([128, 9], dt.float32, tag="loss9")
            nc.vector.tensor_tensor(loss9[:], dotab[:], den[:], op=ALU.mult)
            nc.vector.tensor_scalar(loss9[:], loss9[:], -1.0, 1.0, op0=ALU.mult, op1=ALU.add)
            nc.sync.dma_start(loss_parts[:], loss9[:])

    nc.compile()
    return nc


def kernel(**inputs):
    from concourse import bass_utils
    debug = bool(inputs.pop('_debug', False))
    key = ('nc', debug)
    if key not in _CACHE:
        _CACHE[key] = build_kernel(debug=debug)
    nc = _CACHE[key]
    in_maps = _host_prep(inputs)
    res = bass_utils.run_bass_kernel_spmd(nc, in_maps, core_ids=list(range(N)))
    _CACHE['last_results'] = res.results
    total = np.float64(0.0)
    for k in range(N):
        total += np.asarray(res.results[k]['loss_parts'], np.float64).sum()
    return np.float32(total / 9216.0)


# revision 12
# speedup vs baseline: 1.2310x; 1.2310x over previous
"""Trainium2 Bass kernel for nn_LossStyle (VGG nn-style loss), 8-core SPMD.

Each core computes an eighth (12-row strip of the 96x96 grid, halo recompute)
of VGG features for BOTH images (bf16 activations, fp32 conv3 outputs);
AllGathers: style channel sums; centered+normalized style features (bf16);
per-style-column stats (fp32); output channel sums. Each core then computes
its [1152, 9216] similarity slab on TensorE (pre-normalized operands so PSUM
holds the argmax objective), argmax via max/max_index, indirect-DMA stat
gather, and reconstructs the cosine loss from per-column scalars.
"""
import numpy as np
import ml_dtypes

N = 8  # cores
BF16 = ml_dtypes.bfloat16

MEAN = np.array([0.485, 0.456, 0.406], dtype=np.float32)
STD = np.array([0.229, 0.224, 0.225], dtype=np.float32)

_CACHE = {}


def _host_prep(inputs):
    f32 = np.float32
    w0n = np.asarray(inputs['w0'], f32)
    b0n = np.asarray(inputs['b0'], f32)

    def t9(w):  # [O, C, 3, 3] -> [9, C, O]
        return np.ascontiguousarray(w.transpose(2, 3, 1, 0).reshape(9, w.shape[1], w.shape[0]))

    w1bd = np.zeros((54, 128), f32)
    for t in range(9):
        kh, kw = divmod(t, 3)
        for c in range(3):
            w1bd[t * 3 + c, 0:64] = w0n[:, c, kh, kw]
            w1bd[27 + t * 3 + c, 64:128] = w0n[:, c, kh, kw]

    w1r = t9(np.asarray(inputs['w1'], f32))
    w2bd = np.zeros((128, 9, 128), f32)
    w2bd[0:64, :, 0:64] = w1r.transpose(1, 0, 2)
    w2bd[64:128, :, 64:128] = w1r.transpose(1, 0, 2)

    w2r = t9(np.asarray(inputs['w2'], f32))
    w3bd = np.zeros((128, 9, 256), f32)
    w3bd[0:64, :, 0:128] = w2r.transpose(1, 0, 2)
    w3bd[64:128, :, 128:256] = w2r.transpose(1, 0, 2)

    w4bd = np.ascontiguousarray(t9(np.asarray(inputs['w3'], f32)).transpose(1, 0, 2))
    w5bd = np.ascontiguousarray(t9(np.asarray(inputs['w4'], f32)).transpose(1, 0, 2))

    def big(wref):  # [256, 256, 3, 3] -> [128, 9, 2, 256]
        w = t9(np.asarray(wref, f32))
        out = np.zeros((128, 9, 2, 256), f32)
        for kb in range(2):
            out[:, :, kb, :] = w[:, kb * 128:(kb + 1) * 128, :].transpose(1, 0, 2)
        return out

    w6bd = big(inputs['w5'])
    w7bd = big(inputs['w6'])

    b1 = np.concatenate([b0n, b0n]).astype(f32)
    b1_ = np.asarray(inputs['b1'], f32)
    b2 = np.concatenate([b1_, b1_]).astype(f32)

    weights = dict(
        w1bd=w1bd.astype(BF16), w2bd=w2bd.astype(BF16), w3bd=w3bd.astype(BF16),
        w4bd=w4bd.astype(BF16), w5bd=w5bd.astype(BF16), w6bd=w6bd, w7bd=w7bd,
        bias1=b1, bias2=b2,
        bias3=np.asarray(inputs['b2'], f32), bias4=np.asarray(inputs['b3'], f32),
        bias5=np.asarray(inputs['b4'], f32).reshape(2, 128).T.copy(),
        bias6=np.asarray(inputs['b5'], f32).reshape(2, 128).T.copy(),
        bias7=np.asarray(inputs['b6'], f32).reshape(2, 128).T.copy())

    def slabs(img):
        imgn = (img - MEAN[:, None, None]) / STD[:, None, None]
        out = []
        for k in range(N):
            s = np.zeros((3, 86, 386), f32)
            g0 = 48 * k - 19
            lo, hi = max(0, g0), min(384, g0 + 86)
            s[:, lo - g0:hi - g0, 1:385] = imgn[:, lo:hi, :]
            out.append(s.astype(BF16))
        return out

    def masks_for_core(k):
        def mk(rows, gfun, lim):
            m = np.zeros((128, rows), f32)
            for half in range(2):
                for u in range(rows):
                    g = gfun(u, half)
                    m[64*half:64*half+64, u] = 1.0 if 0 <= g < lim else 0.0
            return m.astype(BF16)
        m44 = mk(44, lambda u, h: 48*k - 18 + (u - 1) + 42*h, 384)
        m42 = mk(42, lambda u, h: 48*k - 18 + u + 42*h, 384)
        m42b = np.zeros((128, 42), f32)
        for u in range(42):
            m42b[:, u] = 1.0 if 0 <= (24*k - 8 + u - 1) < 192 else 0.0
        m40 = np.zeros((128, 40), f32)
        for u in range(40):
            m40[:, u] = 1.0 if 0 <= (24*k - 8 + u) < 192 else 0.0
        m22 = np.zeros((128, 22), f32)
        for u in range(22):
            m22[:, u] = 1.0 if 0 <= (12*k - 4 + u - 1) < 96 else 0.0
        return dict(m44=m44, m42=m42, m42b=m42b.astype(BF16),
                    m40=m40.astype(BF16), m22=m22.astype(np.float32))

    xs = slabs(np.asarray(inputs['outputs'], f32)[0])
    ss = slabs(np.asarray(inputs['styles'], f32)[0])
    in_maps = []
    for k in range(N):
        m = dict(weights)
        m['xslab'] = xs[k]
        m['sslab'] = ss[k]
        m.update(masks_for_core(k))
        in_maps.append(m)
    return in_maps


def build_kernel(debug=False):
    import concourse.bass as bass
    import concourse.bacc as bacc
    import concourse.mybir as mybir
    import concourse.tile as tile

    dt = mybir.dt
    ALU = mybir.AluOpType
    ACT = mybir.ActivationFunctionType
    AX = mybir.AxisListType

    nc = bacc.Bacc("TRN2", target_bir_lowering=False, debug=False, num_devices=N)

    xslab = nc.dram_tensor("xslab", [3, 86, 386], dt.bfloat16, kind="ExternalInput")
    sslab = nc.dram_tensor("sslab", [3, 86, 386], dt.bfloat16, kind="ExternalInput")
    wd = {}
    wd[1] = nc.dram_tensor("w1bd", [54, 128], dt.bfloat16, kind="ExternalInput")
    wd[2] = nc.dram_tensor("w2bd", [128, 9, 128], dt.bfloat16, kind="ExternalInput")
    wd[3] = nc.dram_tensor("w3bd", [128, 9, 256], dt.bfloat16, kind="ExternalInput")
    wd[4] = nc.dram_tensor("w4bd", [128, 9, 128], dt.bfloat16, kind="ExternalInput")
    wd[5] = nc.dram_tensor("w5bd", [128, 9, 256], dt.bfloat16, kind="ExternalInput")
    wd[6] = nc.dram_tensor("w6bd", [128, 9, 2, 256], dt.float32, kind="ExternalInput")
    wd[7] = nc.dram_tensor("w7bd", [128, 9, 2, 256], dt.float32, kind="ExternalInput")
    bias_d = {}
    for i, shp in ((1, [128]), (2, [128]), (3, [128]), (4, [128]),
                   (5, [128, 2]), (6, [128, 2]), (7, [128, 2])):
        bias_d[i] = nc.dram_tensor(f"bias{i}", shp, dt.float32, kind="ExternalInput")

    mask_d = {}
    for nm, rows, mdt in (("m44", 44, dt.bfloat16), ("m42", 42, dt.bfloat16),
                          ("m42b", 42, dt.bfloat16), ("m40", 40, dt.bfloat16),
                          ("m22", 22, dt.float32)):
        mask_d[nm] = nc.dram_tensor(nm, [128, rows], mdt, kind="ExternalInput")

    loss_parts = nc.dram_tensor("loss_parts", [128, 9], dt.float32, kind="ExternalOutput")
    idx_out = nc.dram_tensor("idx_out", [128, 9], dt.uint32, kind="ExternalOutput")
    if debug:
        dbg_feats = nc.dram_tensor("dbg_feats", [128, 6912], dt.float32, kind="ExternalOutput")
        dbg_maxv = nc.dram_tensor("dbg_maxv", [128, 9], dt.float32, kind="ExternalOutput")
        dbg_stat = nc.dram_tensor("dbg_stat", [128, 27], dt.float32, kind="ExternalOutput")

    RG = [list(range(N))]
    BD = dt.bfloat16

    with tile.TileContext(nc) as tc:
        with tc.tile_pool(name="sb", bufs=1) as sb, \
             tc.tile_pool(name="wsl", bufs=2) as wsl, \
             tc.tile_pool(name="cv", bufs=1) as cv, \
             tc.tile_pool(name="sh", bufs=2) as sh, \
             tc.tile_pool(name="pp", bufs=2) as pp, \
             tc.tile_pool(name="ps", bufs=4, space="PSUM") as ps, \
             tc.tile_pool(name="ps2", bufs=1, space="PSUM") as ps2, \
             tc.tile_pool(name="dram", bufs=1, space="DRAM") as dram:

            ones = sb.tile([128, 128], dt.float32, tag="ones")
            nc.vector.memset(ones[:], 1.0)
            zero1 = sb.tile([128, 1], dt.float32, tag="zero1")
            nc.vector.memset(zero1[:], 0.0)
            eps1 = sb.tile([128, 1], dt.float32, tag="eps1")
            nc.vector.memset(eps1[:], 1e-8)
            partS = sb.tile([128, 6], dt.float32, tag="partS")
            partX = sb.tile([128, 6], dt.float32, tag="partX")

            biases = {}
            for i in range(1, 8):
                shp = [128, 2] if i >= 5 else [128, 1]
                t = sb.tile(shp, dt.float32, tag=f"b{i}", name=f"b{i}")
                nc.sync.dma_start(t[:], bias_d[i][:])
                biases[i] = t

            msk = {}
            for nm, rows, mdt in (("m44", 44, dt.bfloat16), ("m42", 42, dt.bfloat16),
                                  ("m42b", 42, dt.bfloat16), ("m40", 40, dt.bfloat16),
                                  ("m22", 22, dt.float32)):
                t = sb.tile([128, rows, 1], mdt, tag=nm, name=nm)
                nc.sync.dma_start(t[:, :, 0], mask_d[nm][:])
                msk[nm] = t

            state = {}

            def conv_image(slab_dram, im, part_tile):
                i2c = cv.tile([54, 42, 384], BD, tag="A", name="i2c")
                for t in range(9):
                    kh, kw = divmod(t, 3)
                    for half in range(2):
                        src = slab_dram[:, kh + 42 * half: kh + 42 * half + 42, kw:kw + 384]
                        nc.sync.dma_start(i2c[27 * half + 3 * t: 27 * half + 3 * t + 3, :, :], src)
                w1 = wsl.tile([54, 128], BD, tag="w", name="w1")
                nc.sync.dma_start(w1[:], wd[1][:])

                c11 = cv.tile([128, 44, 386], BD, tag="B", name="c11")
                nc.vector.memset(c11[0:64, 0:1, :], 0.0)
                nc.vector.memset(c11[64:128, 43:44, :], 0.0)
                nc.vector.memset(c11[:, :, 0:1], 0.0)
                nc.vector.memset(c11[:, :, 385:386], 0.0)
                for r in range(42):
                    p = ps.tile([128, 384], dt.float32, tag="mm", name="p11")
                    nc.tensor.matmul(p[:], w1[:], i2c[:, r, :], start=True, stop=True)
                    nc.scalar.activation(c11[:, r + 1, 1:385], p[:], ACT.Relu,
                                         bias=biases[1][:, 0:1])
                nc.vector.tensor_tensor(c11[:], c11[:], msk["m44"][:].to_broadcast([128, 44, 386]),
                                        op=ALU.mult)
                nc.sync.dma_start(c11[0:64, 43:44, :], c11[64:128, 1:2, :])
                nc.sync.dma_start(c11[64:128, 0:1, :], c11[0:64, 42:43, :])

                w2 = wsl.tile([128, 9, 128], BD, tag="w", name="w2")
                nc.sync.dma_start(w2[:], wd[2][:])
                c12 = cv.tile([128, 42, 386], BD, tag="A", name="c12")
                nc.vector.memset(c12[:, :, 0:1], 0.0)
                nc.vector.memset(c12[:, :, 385:386], 0.0)
                for v in range(42):
                    p = ps.tile([128, 384], dt.float32, tag="mm", name="p12")
                    for t in range(9):
                        kh, kw = divmod(t, 3)
                        nc.tensor.matmul(p[:], w2[:, t, :], c11[:, v + kh, kw:kw + 384],
                                         start=(t == 0), stop=(t == 8))
                    nc.scalar.activation(c12[:, v, 1:385], p[:], ACT.Relu,
                                         bias=biases[2][:, 0:1])

                nc.vector.tensor_tensor(c12[:], c12[:], msk["m42"][:].to_broadcast([128, 42, 386]),
                                        op=ALU.mult)
                p1t = cv.tile([128, 42, 192], BD, tag="B", name="p1t")
                nc.vector.tensor_tensor(p1t[:], c12[:, :, 1:385:2], c12[:, :, 2:386:2], op=ALU.max)
                p1o = cv.tile([128, 21, 192], BD, tag="A", name="p1o")
                nc.vector.tensor_tensor(p1o[:], p1t[:, 0:42:2, :], p1t[:, 1:42:2, :], op=ALU.max)

                c21 = cv.tile([128, 22, 194], BD, tag="B", name="c21")
                nc.vector.memset(c21[:, :, 0:1], 0.0)
                nc.vector.memset(c21[:, :, 193:194], 0.0)
                nc.vector.tensor_copy(c21[0:64, 0:21, 1:193], p1o[0:64, :, :])
                nc.vector.tensor_copy(c21[64:128, 1:22, 1:193], p1o[64:128, :, :])
                nc.sync.dma_start(c21[0:64, 21:22, 1:193], p1o[64:128, 0:1, :])
                nc.sync.dma_start(c21[64:128, 0:1, 1:193], p1o[0:64, 20:21, :])

                w3 = wsl.tile([128, 9, 256], BD, tag="w", name="w3")
                nc.sync.dma_start(w3[:], wd[3][:])
                c22i = cv.tile([128, 42, 194], BD, tag="A", name="c22i")
                nc.vector.memset(c22i[:, 0:1, :], 0.0)
                nc.vector.memset(c22i[:, 41:42, :], 0.0)
                nc.vector.memset(c22i[:, :, 0:1], 0.0)
                nc.vector.memset(c22i[:, :, 193:194], 0.0)
                for mb in range(2):
                    for q0 in range(0, 20, 2):
                        p = ps.tile([128, 2, 192], dt.float32, tag="mm", name="p21")
                        for t in range(9):
                            kh, kw = divmod(t, 3)
                            nc.tensor.matmul(p[:], w3[:, t, 128 * mb:128 * mb + 128],
                                             c21[:, q0 + kh:q0 + kh + 2, kw:kw + 192],
                                             start=(t == 0), stop=(t == 8))
                        w_ = 20 * mb + q0 + 1
                        nc.scalar.activation(c22i[:, w_:w_ + 2, 1:193], p[:], ACT.Relu,
                                             bias=biases[3][:, 0:1])

                nc.vector.tensor_tensor(c22i[:], c22i[:], msk["m42b"][:].to_broadcast([128, 42, 194]),
                                        op=ALU.mult)
                w4 = wsl.tile([128, 9, 128], BD, tag="w", name="w4")
                nc.sync.dma_start(w4[:], wd[4][:])
                c22o = cv.tile([128, 40, 194], BD, tag="B", name="c22o")
                nc.vector.memset(c22o[:, :, 0:1], 0.0)
                nc.vector.memset(c22o[:, :, 193:194], 0.0)
                for q0 in range(0, 40, 2):
                    p = ps.tile([128, 2, 192], dt.float32, tag="mm", name="p22")
                    for t in range(9):
                        kh, kw = divmod(t, 3)
                        nc.tensor.matmul(p[:], w4[:, t, :],
                                         c22i[:, q0 + kh:q0 + kh + 2, kw:kw + 192],
                                         start=(t == 0), stop=(t == 8))
                    nc.scalar.activation(c22o[:, q0:q0 + 2, 1:193], p[:], ACT.Relu,
                                         bias=biases[4][:, 0:1])

                nc.vector.tensor_tensor(c22o[:], c22o[:], msk["m40"][:].to_broadcast([128, 40, 194]),
                                        op=ALU.mult)
                p2t = cv.tile([128, 40, 96], BD, tag="A", name="p2t")
                nc.vector.tensor_tensor(p2t[:], c22o[:, :, 1:193:2], c22o[:, :, 2:194:2], op=ALU.max)
                c31 = cv.tile([128, 22, 98], BD, tag="B", name="c31")
                nc.vector.memset(c31[:, 0:1, :], 0.0)
                nc.vector.memset(c31[:, 21:22, :], 0.0)
                nc.vector.memset(c31[:, :, 0:1], 0.0)
                nc.vector.memset(c31[:, :, 97:98], 0.0)
                nc.vector.tensor_tensor(c31[:, 1:21, 1:97], p2t[:, 0:40:2, :],
                                        p2t[:, 1:40:2, :], op=ALU.max)

                w5 = wsl.tile([128, 9, 256], BD, tag="w", name="w5")
                nc.sync.dma_start(w5[:], wd[5][:])
                f11 = cv.tile([128, 2, 22, 98], dt.float32, tag="C", name="f11")
                for kb in range(2):
                    nc.vector.memset(f11[:, kb, 0:1, :], 0.0)
                    nc.vector.memset(f11[:, kb, 21:22, :], 0.0)
                    nc.vector.memset(f11[:, kb, :, 0:1], 0.0)
                    nc.vector.memset(f11[:, kb, :, 97:98], 0.0)
                for mb in range(2):
                    for h0 in range(0, 20, 5):
                        p = ps.tile([128, 5, 96], dt.float32, tag="mm", name="p31")
                        for t in range(9):
                            kh, kw = divmod(t, 3)
                            nc.tensor.matmul(p[:], w5[:, t, 128 * mb:128 * mb + 128],
                                             c31[:, h0 + kh:h0 + kh + 5, kw:kw + 96],
                                             start=(t == 0), stop=(t == 8))
                        nc.scalar.activation(f11[:, mb, h0 + 1:h0 + 6, 1:97], p[:], ACT.Relu,
                                             bias=biases[5][:, mb:mb + 1])

                for kb in range(2):
                    nc.vector.tensor_tensor(f11[:, kb], f11[:, kb],
                                            msk["m22"][:].to_broadcast([128, 22, 98]), op=ALU.mult)
                w6 = wsl.tile([128, 9, 2, 256], dt.float32, tag="w", name="w6")
                nc.sync.dma_start(w6[:], wd[6][:])
                f13 = cv.tile([128, 2, 22, 98], dt.float32, tag="A", name="f13")
                for kb in range(2):
                    nc.vector.memset(f13[:, kb, 0:1, :], 0.0)
                    nc.vector.memset(f13[:, kb, 21:22, :], 0.0)
                    nc.vector.memset(f13[:, kb, :, 0:1], 0.0)
                    nc.vector.memset(f13[:, kb, :, 97:98], 0.0)
                for mb in range(2):
                    for h0 in range(0, 20, 5):
                        p = ps.tile([128, 5, 96], dt.float32, tag="mm", name="p32")
                        first = True
                        for kb in range(2):
                            for t in range(9):
                                kh, kw = divmod(t, 3)
                                nc.tensor.matmul(p[:], w6[:, t, kb, 128 * mb:128 * mb + 128],
                                                 f11[:, kb, h0 + kh:h0 + kh + 5, kw:kw + 96],
                                                 start=first, stop=(kb == 1 and t == 8))
                                first = False
                        nc.scalar.activation(f13[:, mb, h0 + 1:h0 + 6, 1:97], p[:], ACT.Relu,
                                             bias=biases[6][:, mb:mb + 1])

                for kb in range(2):
                    nc.vector.tensor_tensor(f13[:, kb], f13[:, kb],
                                            msk["m22"][:].to_broadcast([128, 22, 98]), op=ALU.mult)
                w7 = wsl.tile([128, 9, 2, 256], dt.float32, tag="w", name="w7")
                nc.sync.dma_start(w7[:], wd[7][:])
                f15 = cv.tile([128, 2, 12, 96], dt.float32, tag="B", name="f15")
                for mb in range(2):
                    for h0, nr in ((4, 5), (9, 5), (14, 2)):
                        p = ps.tile([128, 5, 96], dt.float32, tag="mm", name="p33")
                        first = True
                        for kb in range(2):
                            for t in range(9):
                                kh, kw = divmod(t, 3)
                                nc.tensor.matmul(p[:, 0:nr, :],
                                                 w7[:, t, kb, 128 * mb:128 * mb + 128],
                                                 f13[:, kb, h0 + kh:h0 + kh + nr, kw:kw + 96],
                                                 start=first, stop=(kb == 1 and t == 8))
                                first = False
                        nc.scalar.activation(f15[:, mb, h0 - 4:h0 - 4 + nr, :], p[:, 0:nr, :],
                                             ACT.Relu, bias=biases[7][:, mb:mb + 1])

                wins = {0: f11[:, 0, 5:17, 1:97], 1: f11[:, 1, 5:17, 1:97],
                        2: f13[:, 0, 5:17, 1:97], 3: f13[:, 1, 5:17, 1:97],
                        4: f15[:, 0, :, :], 5: f15[:, 1, :, :]}
                if im == 0:
                    feat0 = sh.tile([128, 6912], dt.float32, tag="shA", name="feat0")
                    state['feat0'] = feat0
                    for b in range(6):
                        nc.vector.tensor_copy(
                            feat0[:, b * 1152:(b + 1) * 1152].rearrange("p (r x) -> p r x", r=12),
                            wins[b])
                        nc.vector.tensor_reduce(part_tile[:, b:b + 1],
                                                feat0[:, b * 1152:(b + 1) * 1152],
                                                axis=AX.X, op=ALU.add)
                else:
                    state['xwin'] = wins
                    for b in range(6):
                        nc.vector.tensor_reduce(part_tile[:, b:b + 1], wins[b],
                                                axis=AX.XY, op=ALU.add)

            # ======== styles conv, style AGs, then x conv (overlaps AG2) ========
            conv_image(sslab, 0, partS)

            cc1s_in = dram.tile([128, 6], dt.float32)
            cc1s_out = dram.tile([128 * N, 6], dt.float32, addr_space="Shared")
            nc.sync.dma_start(cc1s_in[:], partS[:])
            nc.gpsimd.collective_compute("AllGather", ALU.bypass, replica_groups=RG,
                                         ins=[cc1s_in.opt()], outs=[cc1s_out.opt()])
            sumS = sb.tile([128, 48], dt.float32, tag="sums", name="sumS")
            nc.sync.dma_start(sumS[:].rearrange("p (r c) -> p r c", r=8),
                              cc1s_out[:].rearrange("(r p) c -> p r c", p=128))
            bmean = sb.tile([128, 6], dt.float32, tag="bmean")
            for col in range(6):
                nc.vector.tensor_reduce(bmean[:, col:col + 1], sumS[:, col:48:6],
                                        axis=AX.X, op=ALU.add)
            nc.scalar.mul(bmean[:], bmean[:], 1.0 / 9216.0)

            Fs = state['feat0']
            for b in range(6):
                nc.vector.tensor_scalar(Fs[:, b * 1152:(b + 1) * 1152],
                                        Fs[:, b * 1152:(b + 1) * 1152],
                                        bmean[:, b:b + 1], None, op0=ALU.subtract)
            nbc_t = sb.tile([128, 1152], dt.float32, tag="nbc_u", name="nbc_t")
            inv_t = sb.tile([128, 1152], dt.float32, tag="inv_na", name="inv_t")
            cc3_in = dram.tile([1152, 3], dt.float32)
            cc3_out = dram.tile([1152 * N, 3], dt.float32, addr_space="Shared")
            for ch in range(3):
                c0 = ch * 384
                p_s2 = ps2.tile([128, 384], dt.float32, tag="p_s2", name="ps2s")
                p_pm = ps2.tile([128, 384], dt.float32, tag="p_pm", name="ppms")
                for b in range(6):
                    sq = pp.tile([128, 384], dt.float32, tag="sq", bufs=1, name="sqs")
                    nc.scalar.activation(sq[:], Fs[:, b * 1152 + c0:b * 1152 + c0 + 384],
                                         ACT.Square, bias=zero1[:])
                    rep = pp.tile([128, 128], dt.float32, tag="rep", name="reps")
                    nc.vector.tensor_scalar(rep[:], ones[:], bmean[:, b:b + 1], None, op0=ALU.mult)
                    nc.tensor.matmul(p_s2[:], ones[:], sq[:], start=(b == 0), stop=(b == 5))
                    nc.tensor.matmul(p_pm[:], rep[:], Fs[:, b * 1152 + c0:b * 1152 + c0 + 384],
                                     start=(b == 0), stop=(b == 5))
                nc.scalar.activation(nbc_t[:, c0:c0 + 384], p_s2[:], ACT.Sqrt, bias=eps1[:])
                nc.vector.tensor_scalar(nbc_t[:, c0:c0 + 384], nbc_t[:, c0:c0 + 384],
                                        1e-8, None, op0=ALU.add)
                nc.vector.reciprocal(inv_t[:, c0:c0 + 384], nbc_t[:, c0:c0 + 384])
                pmcp = pp.tile([128, 384], dt.float32, tag="sq", bufs=1, name="pmcp")
                nc.scalar.copy(pmcp[:], p_pm[:])
                nc.sync.dma_start(cc3_in[c0:c0 + 384, 1:2], pmcp[0:1, :])
            nc.sync.dma_start(cc3_in[:, 0:1], nbc_t[0:1, :])  # sbuf row -> strided dram

            bhat = sb.tile([128, 6912], dt.bfloat16, tag="cast16", name="bhat")
            for b in range(6):
                nc.vector.tensor_tensor(bhat[:, b * 1152:(b + 1) * 1152],
                                        Fs[:, b * 1152:(b + 1) * 1152], inv_t[:], op=ALU.mult)
            cc2_in = dram.tile([768, 1152], dt.bfloat16)
            cc2_out = dram.tile([768 * N, 1152], dt.bfloat16, addr_space="Shared")
            nc.sync.dma_start(cc2_in[:].rearrange("(b p) x -> p b x", p=128),
                              bhat[:].rearrange("p (b x) -> p b x", b=6))
            nc.gpsimd.collective_compute("AllGather", ALU.bypass, replica_groups=RG,
                                         ins=[cc2_in.opt()], outs=[cc2_out.opt()])

            # ======== x conv (overlaps AG2) ========
            conv_image(xslab, 1, partX)

            cc1x_in = dram.tile([128, 6], dt.float32)
            cc1x_out = dram.tile([128 * N, 6], dt.float32, addr_space="Shared")
            nc.sync.dma_start(cc1x_in[:], partX[:])
            nc.gpsimd.collective_compute("AllGather", ALU.bypass, replica_groups=RG,
                                         ins=[cc1x_in.opt()], outs=[cc1x_out.opt()])
            sumX = sb.tile([128, 48], dt.float32, tag="sums", name="sumX")
            nc.sync.dma_start(sumX[:].rearrange("p (r c) -> p r c", r=8),
                              cc1x_out[:].rearrange("(r p) c -> p r c", p=128))
            amean = sb.tile([128, 6], dt.float32, tag="amean")
            for col in range(6):
                nc.vector.tensor_reduce(amean[:, col:col + 1], sumX[:, col:48:6],
                                        axis=AX.X, op=ALU.add)
            nc.scalar.mul(amean[:], amean[:], 1.0 / 9216.0)

            psc_a = ps2.tile([1, 1], dt.float32, tag="p_s2", name="psc_a")
            psc_b = ps2.tile([1, 1], dt.float32, tag="p_pm", name="psc_b")
            psc_w = ps2.tile([1, 1], dt.float32, tag="p_v", name="psc_w")
            for b in range(6):
                st, sp = (b == 0), (b == 5)
                nc.tensor.matmul(psc_a[:], amean[:, b:b + 1], amean[:, b:b + 1], start=st, stop=sp)
                nc.tensor.matmul(psc_b[:], bmean[:, b:b + 1], bmean[:, b:b + 1], start=st, stop=sp)
                nc.tensor.matmul(psc_w[:], amean[:, b:b + 1], bmean[:, b:b + 1], start=st, stop=sp)
            scal = sb.tile([1, 3], dt.float32, tag="scal")
            nc.scalar.copy(scal[:, 0:1], psc_a[:])
            nc.scalar.copy(scal[:, 1:2], psc_b[:])
            nc.scalar.copy(scal[:, 2:3], psc_w[:])
            pscr = ps2.tile([128, 3], dt.float32, tag="p_s2", name="pscr")
            nc.tensor.matmul(pscr[:], ones[0:1, :], scal[:], start=True, stop=True)
            screp = sb.tile([128, 3], dt.float32, tag="screp")
            nc.scalar.copy(screp[:], pscr[:])  # A2, B2, W

            for ch in range(3):
                c0 = ch * 384
                p_v = ps2.tile([128, 384], dt.float32, tag="p_v", name="pvs")
                for b in range(6):
                    rep = pp.tile([128, 128], dt.float32, tag="rep", name="repv")
                    nc.vector.tensor_scalar(rep[:], ones[:], amean[:, b:b + 1], None, op0=ALU.mult)
                    nc.tensor.matmul(p_v[:], rep[:], Fs[:, b * 1152 + c0:b * 1152 + c0 + 384],
                                     start=(b == 0), stop=(b == 5))
                pvcp = pp.tile([128, 384], dt.float32, tag="sq", bufs=1, name="pvcp")
                nc.scalar.copy(pvcp[:], p_v[:])
                nc.sync.dma_start(cc3_in[c0:c0 + 384, 2:3], pvcp[0:1, :])
            nc.gpsimd.collective_compute("AllGather", ALU.bypass, replica_groups=RG,
                                         ins=[cc3_in.opt()], outs=[cc3_out.opt()])

            # ======== x side: center + stats (windowed) + bf16 cast ========
            xwin = state['xwin']
            for b in range(6):
                nc.vector.tensor_scalar(xwin[b], xwin[b], amean[:, b:b + 1], None,
                                        op0=ALU.subtract)
            acb = sb.tile([128, 6912], dt.bfloat16, tag="cast16", name="acb")
            for b in range(6):
                nc.vector.tensor_copy(
                    acb[:, b * 1152:(b + 1) * 1152].rearrange("p (r x) -> p r x", r=12),
                    xwin[b])
            if debug:
                nc.sync.dma_start(dbg_feats[:], Fs[:])

            na_t = sb.tile([128, 1152], dt.float32, tag="inv_na", name="na_t")
            u_t = sb.tile([128, 1152], dt.float32, tag="nbc_u", name="u_t")
            for ch in range(3):
                c0 = ch * 384
                p_s2 = ps2.tile([128, 384], dt.float32, tag="p_s2", name="ps2x")
                p_pm = ps2.tile([128, 384], dt.float32, tag="p_pm", name="ppmx")
                p_v = ps2.tile([128, 384], dt.float32, tag="p_v", name="pvx")
                for b in range(6):
                    win = xwin[b][:, 4 * ch:4 * ch + 4, :]
                    sq = pp.tile([128, 384], dt.float32, tag="sq", bufs=1, name="sqx")
                    nc.scalar.activation(sq[:], win, ACT.Square, bias=zero1[:])
                    repa = pp.tile([128, 128], dt.float32, tag="rep", name="repa")
                    nc.vector.tensor_scalar(repa[:], ones[:], amean[:, b:b + 1], None, op0=ALU.mult)
                    repb = pp.tile([128, 128], dt.float32, tag="rep", name="repb")
                    nc.vector.tensor_scalar(repb[:], ones[:], bmean[:, b:b + 1], None, op0=ALU.mult)
                    nc.tensor.matmul(p_s2[:], ones[:], sq[:], start=(b == 0), stop=(b == 5))
                    nc.tensor.matmul(p_pm[:], repa[:], win, start=(b == 0), stop=(b == 5))
                    nc.tensor.matmul(p_v[:], repb[:], win, start=(b == 0), stop=(b == 5))
                nc.vector.tensor_scalar(na_t[:, c0:c0 + 384], p_pm[:], 2.0, None, op0=ALU.mult)
                nc.vector.tensor_tensor(na_t[:, c0:c0 + 384], na_t[:, c0:c0 + 384], p_s2[:],
                                        op=ALU.add)
                nc.vector.tensor_scalar(na_t[:, c0:c0 + 384], na_t[:, c0:c0 + 384],
                                        screp[:, 0:1], None, op0=ALU.add)
                nc.scalar.activation(na_t[:, c0:c0 + 384], na_t[:, c0:c0 + 384], ACT.Sqrt,
                                     bias=zero1[:])
                nc.scalar.copy(u_t[:, c0:c0 + 384], p_v[:])

            nat = sb.tile([128, 9], dt.float32, tag="nat")
            ut = sb.tile([128, 9], dt.float32, tag="ut")
            for m in range(9):
                nc.sync.dma_start(nat[:, m:m + 1], na_t[0:1, m * 128:(m + 1) * 128])
                nc.sync.dma_start(ut[:, m:m + 1], u_t[0:1, m * 128:(m + 1) * 128])

            # ======== cdist + argmax (stream bhat by column quarters) ========
            vmall = sb.tile([128, 36], dt.float32, tag="vmall")
            viall = sb.tile([128, 36], dt.float32, tag="viall")
            CH = (0, 512, 1024, 1536, 2048, 2304)
            for q in range(4):
                ballq = sh.tile([128, 6, 2304], dt.bfloat16, tag="shA", name="ballq")
                for r in range(2):
                    rows = (2 * q + r) * 768
                    nc.sync.dma_start(
                        ballq[:, :, r * 1152:(r + 1) * 1152],
                        cc2_out[rows:rows + 768, :].rearrange("(b p) x -> p b x", p=128))
                for m in range(9):
                    simq = wsl.tile([128, 2304], dt.float32, tag="w", name="simq")
                    for ci in range(5):
                        n0, n1 = CH[ci], CH[ci + 1]
                        p = ps.tile([128, 512], dt.float32, tag="mm", name="pcd")
                        for b in range(6):
                            nc.tensor.matmul(
                                p[:, 0:n1 - n0],
                                acb[:, b * 1152 + m * 128:b * 1152 + (m + 1) * 128],
                                ballq[:, b, n0:n1],
                                start=(b == 0), stop=(b == 5))
                        nc.scalar.copy(simq[:, n0:n1], p[:, 0:n1 - n0])
                    vm8 = pp.tile([128, 8], dt.float32, tag="vm8", name="vm8")
                    vi8 = pp.tile([128, 8], dt.uint32, tag="vi8", name="vi8")
                    nc.vector.max(vm8[:], simq[:])
                    nc.vector.max_index(vi8[:], vm8[:], simq[:])
                    nc.vector.tensor_copy(vmall[:, q * 9 + m:q * 9 + m + 1], vm8[:, 0:1])
                    nc.vector.tensor_copy(viall[:, q * 9 + m:q * 9 + m + 1], vi8[:, 0:1])

            best = sb.tile([128, 9], dt.float32, tag="best")
            bidx = sb.tile([128, 9], dt.float32, tag="bidx")
            nc.vector.tensor_copy(best[:], vmall[:, 0:9])
            nc.vector.tensor_copy(bidx[:], viall[:, 0:9])
            mq = sb.tile([128, 9], dt.uint8, tag="mq")
            iq = sb.tile([128, 9], dt.float32, tag="iq")
            for q in range(1, 4):
                nc.vector.tensor_scalar(iq[:], viall[:, q * 9:(q + 1) * 9],
                                        float(q * 2304), None, op0=ALU.add)
                nc.vector.tensor_tensor(mq[:], best[:], vmall[:, q * 9:(q + 1) * 9], op=ALU.is_ge)
                nc.vector.select(bidx[:], mq[:], bidx[:], iq[:])
                nc.vector.tensor_tensor(best[:], best[:], vmall[:, q * 9:(q + 1) * 9], op=ALU.max)
            idxs = sb.tile([128, 9], dt.uint32, tag="idxs")
            nc.vector.tensor_copy(idxs[:], bidx[:])
            nc.sync.dma_start(idx_out[:], idxs[:])

            statall = sb.tile([128, 27], dt.float32, tag="statall")
            for m in range(9):
                nc.gpsimd.indirect_dma_start(
                    out=statall[:, 3 * m:3 * m + 3], out_offset=None,
                    in_=cc3_out[:],
                    in_offset=bass.IndirectOffsetOnAxis(ap=idxs[:, m:m + 1], axis=0))
            if debug:
                nc.sync.dma_start(dbg_maxv[:], best[:])
                nc.sync.dma_start(dbg_stat[:], statall[:])

            # ======== loss assembly ========
            nbc_g = statall[:, 0:27:3]
            pmb_g = statall[:, 1:27:3]
            v_g = statall[:, 2:27:3]
            dotab = sb.tile([128, 9], dt.float32, tag="dotab")
            nc.vector.tensor_tensor(dotab[:], best[:], nbc_g, op=ALU.mult)
            nc.vector.tensor_tensor(dotab[:], dotab[:], ut[:], op=ALU.add)
            nc.vector.tensor_tensor(dotab[:], dotab[:], v_g, op=ALU.add)
            nc.vector.tensor_scalar(dotab[:], dotab[:], screp[:, 2:3], None, op0=ALU.add)
            nb = sb.tile([128, 9], dt.float32, tag="nb")
            nc.vector.tensor_scalar(nb[:], nbc_g, -1e-8, None, op0=ALU.add)
            nc.vector.tensor_tensor(nb[:], nb[:], nb[:], op=ALU.mult)
            nc.vector.tensor_scalar(nb[:], nb[:], -1e-8, None, op0=ALU.add)
            tmp9 = sb.tile([128, 9], dt.float32, tag="tmp9")
            nc.vector.tensor_scalar(tmp9[:], pmb_g, 2.0, None, op0=ALU.mult)
            nc.vector.tensor_tensor(nb[:], nb[:], tmp9[:], op=ALU.add)
            nc.vector.tensor_scalar(nb[:], nb[:], screp[:, 1:2], None, op0=ALU.add)
            nc.scalar.activation(nb[:], nb[:], ACT.Sqrt, bias=zero1[:])
            den = sb.tile([128, 9], dt.float32, tag="den")
            nc.vector.tensor_scalar(den[:], nat[:], 1e-8, None, op0=ALU.add)
            nc.vector.tensor_scalar(nb[:], nb[:], 1e-8, None, op0=ALU.add)
            nc.vector.tensor_tensor(den[:], den[:], nb[:], op=ALU.mult)
            nc.vector.reciprocal(den[:], den[:])
            loss9 = sb.tile([128, 9], dt.float32, tag="loss9")
            nc.vector.tensor_tensor(loss9[:], dotab[:], den[:], op=ALU.mult)
            nc.vector.tensor_scalar(loss9[:], loss9[:], -1.0, 1.0, op0=ALU.mult, op1=ALU.add)
            nc.sync.dma_start(loss_parts[:], loss9[:])

    nc.compile()
    return nc


def kernel(**inputs):
    from concourse import bass_utils
    debug = bool(inputs.pop('_debug', False))
    key = ('nc', debug)
    if key not in _CACHE:
        _CACHE[key] = build_kernel(debug=debug)
    nc = _CACHE[key]
    in_maps = _host_prep(inputs)
    res = bass_utils.run_bass_kernel_spmd(nc, in_maps, core_ids=list(range(N)))
    _CACHE['last_results'] = res.results
    total = np.float64(0.0)
    for k in range(N):
        total += np.asarray(res.results[k]['loss_parts'], np.float64).sum()
    return np.float32(total / 9216.0)
